# revision 36
# baseline (speedup 1.0000x reference)
"""Expert-parallel MoE (8 experts, top-2, D=768, H=3072, N=2048) on 8 trn2 cores.

v2 design (vs baseline):
- Routing is computed fully locally on every core (replicated) -- the mid-kernel
  AllGather and its barrier serialization are gone; the only collective is the
  final ReduceScatter, whose stream-init barrier overlaps local compute.
- MLP runs in bf16 (host-cast weights + on-chip x cast); ReduceScatter payload
  is bf16 (halves collective bytes).
- Token dispatch (compaction) is a one-hot matmul: PT[t, slot] = (pos[t]==slot)
  built with iota/is_eq; compact xT = sum_t x[t,:]^T PT[t,:]. No indirect-DMA
  scatter tables.  Slot->token index + gate are recovered with one small matmul
  chain against PT, so only the 5 y-row scatters use indirect DMA.
"""
import numpy as np
import ml_dtypes

import concourse.bass as bass
import concourse.tile as tile
import concourse.mybir as mybir
from concourse import bacc
from concourse.bass_utils import run_bass_kernel_spmd
from concourse.masks import make_identity, make_upper_triangular

F32 = mybir.dt.float32
F32R = mybir.dt.float32r
BF16 = mybir.dt.bfloat16
I32 = mybir.dt.int32
AF = mybir.ActivationFunctionType
ALU = mybir.AluOpType

N_CORES = 8
CORE_IDS = list(range(N_CORES))

N = 2048            # tokens
D = 768             # d_model
H = 3072            # d_ff
E = 8               # experts
NS = N // N_CORES   # output tokens per core (256)
CAP = 640           # per-expert token capacity (max observed load 557)
NT = N // 128       # 16 token tiles
DC = D // 128       # 6 d chunks
HC = H // 128       # 24 h chunks
BIG = float(1 << 20)


def build():
    nc = bacc.Bacc("TRN2", target_bir_lowering=False, debug=False,
                   num_devices=N_CORES)

    x = nc.dram_tensor("x", [N, D], F32, kind="ExternalInput").ap()
    x2 = nc.dram_tensor("x2", [N, D], BF16, kind="ExternalInput").ap()
    rwt = nc.dram_tensor("rwt", [D, E], F32, kind="ExternalInput").ap()
    w1 = nc.dram_tensor("w1", [D, H], BF16, kind="ExternalInput").ap()
    w2 = nc.dram_tensor("w2", [H, D], BF16, kind="ExternalInput").ap()
    esel = nc.dram_tensor("esel", [128, E], F32, kind="ExternalInput").ap()
    out = nc.dram_tensor("out", [NS, D], F32, kind="ExternalOutput").ap()

    from contextlib import ExitStack
    with tile.TileContext(nc) as tc, ExitStack() as ctx:
        sb = ctx.enter_context(tc.tile_pool(name="sb", bufs=1))
        psA = ctx.enter_context(tc.tile_pool(name="psA", bufs=3, space="PSUM"))
        ps1 = ctx.enter_context(tc.tile_pool(name="ps1", bufs=3, space="PSUM"))
        ps2 = ctx.enter_context(tc.tile_pool(name="ps2", bufs=2, space="PSUM"))
        xsp = ctx.enter_context(tc.tile_pool(name="xsp", bufs=4))
        xtp = ctx.enter_context(tc.tile_pool(name="xtp", bufs=1))
        dr = ctx.enter_context(tc.tile_pool(name="dr", bufs=1, space="DRAM"))

        # ---------------- DRAM scratch ----------------
        # A2A buffers: 8 dest blocks x 128-slot capacity; row = 768 y cols +
        # idx-within-dest-range (col 768) + 7 pad cols.
        CAPD = 128                   # per (expert, dest-range) capacity
        AW = 776                     # a2a row width (bf16)
        a2a_send = dr.tile([N_CORES * CAPD, AW], BF16)
        a2a_recv = dr.tile([N_CORES * CAPD, AW], BF16)
        warm_in = dr.tile([8, 32], BF16)
        warm_out = dr.tile([64, 32], BF16, addr_space="Shared")

        # ---------------- constants ----------------
        ident = sb.tile([128, 128], F32)
        make_identity(nc, ident[:])
        identb = sb.tile([128, 128], BF16)
        make_identity(nc, identb[:])
        uincl = sb.tile([128, 128], F32)   # [q <= p] as lhsT: incl prefix
        make_upper_triangular(nc, uincl[:], val=1.0, diag=True)
        ones1 = sb.tile([1, 128], F32)
        nc.vector.memset(ones1[:], 1.0)
        iota640i = sb.tile([128, CAP], I32)
        nc.gpsimd.iota(iota640i[:], pattern=[[1, CAP]], base=0,
                       channel_multiplier=0)
        iota640f = sb.tile([128, CAP], F32)
        nc.vector.tensor_copy(iota640f[:], iota640i[:])
        fvals_i = sb.tile([128, NT], I32)   # col f -> f
        nc.gpsimd.iota(fvals_i[:], pattern=[[1, NT]], base=0,
                       channel_multiplier=0)
        pvals_i = sb.tile([128, NT], I32)   # value p everywhere
        nc.gpsimd.iota(pvals_i[:], pattern=[[0, NT]], base=0,
                       channel_multiplier=1)
        esel_sb = sb.tile([128, E], F32)
        nc.sync.dma_start(out=esel_sb[:], in_=esel[:])
        zero_row = sb.tile([128, D], BF16)
        nc.vector.memset(zero_row[:], 0.0)
        rwt_sb = [sb.tile([128, E], F32, name=f"rwt_sb{d}") for d in range(DC)]
        for d in range(DC):
            nc.scalar.dma_start(out=rwt_sb[d][:],
                                in_=rwt[d * 128:(d + 1) * 128, :])

        # ---------------- early bulk DMAs ----------------
        # x f32 (critical path head) + x bf16 on sync HW queue
        xs_t = [xsp.tile([128, D], F32, name="xs", tag="xs") for _ in range(NT)]
        for t in range(NT):
            nc.sync.dma_start(out=xs_t[t][:], in_=x[t * 128:(t + 1) * 128, :])
        # w1 on scalar HW queue (needed at ~t60); w2 is issued later in the
        # scalar program order (after routing's exp calls) so it doesn't
        # compete with x/w1 for DMA engines at the head.
        w1sb = [sb.tile([128, H], BF16, name=f"w1sb{d}") for d in range(DC)]
        for d in range(DC):
            nc.scalar.dma_start(out=w1sb[d][:],
                                in_=w1[d * 128:(d + 1) * 128, :])
        w2sb = [sb.tile([128, D], BF16, name=f"w2sb{h}") for h in range(HC)]
        # zero-init a2a send buffer early on the gpsimd queue (the dummy
        # stream-warming AllGather fires later, after the token gathers)
        nc.gpsimd.dma_start(out=warm_in[:], in_=zero_row[0:8, 0:32])
        zero_aw = sb.tile([128, AW], BF16)
        nc.vector.memset(zero_aw[:], 0.0)
        for t in range(N_CORES * CAPD // 128):
            nc.gpsimd.dma_start(out=a2a_send[t * 128:(t + 1) * 128, :],
                                in_=zero_aw[:])

        # ---------------- routing (all 2048 tokens, f32) ----------------
        ssum = sb.tile([128, NT], F32)
        graw = sb.tile([128, NT], F32)
        for g in range(4):          # groups of 4 token tiles (512 tokens)
            xT = [xtp.tile([128, 512], F32, name="xT", tag=f"xT{d}")
                  for d in range(DC)]
            for d in range(DC):
                ptg = psA.tile([128, 512], F32, name="ptg", tag="pA")
                for t in range(4):
                    nc.tensor.transpose(
                        ptg[:, t * 128:(t + 1) * 128],
                        xs_t[4 * g + t][:, d * 128:(d + 1) * 128], ident[:])
                nc.vector.tensor_copy(xT[d][:], ptg[:])
            pl = psA.tile([8, 512], F32, name="pl", tag="pA")
            for d in range(DC):
                nc.tensor.matmul(pl[:], lhsT=rwt_sb[d][:], rhs=xT[d][:],
                                 start=(d == 0), stop=(d == DC - 1))
            l_sb = sb.tile([8, 512], F32, name="l_sb", tag="l_sb", bufs=2)
            nc.vector.tensor_copy(l_sb[:], pl[:])
            for t in range(4):
                tc_i = 4 * g + t
                ptl = psA.tile([128, 8], F32, name="ptl", tag="pA")
                nc.tensor.transpose(ptl[:], l_sb[:, t * 128:(t + 1) * 128],
                                    ident[:8, :8])
                lg = sb.tile([128, 8], F32, name="lg", tag="lg", bufs=4)
                nc.vector.tensor_copy(lg[:], ptl[:])
                srt = sb.tile([128, 8], F32, name="srt", tag="srt", bufs=4)
                nc.vector.max(srt[:], lg[:])
                negm = sb.tile([128, 1], F32, name="negm", tag="negm", bufs=4)
                nc.vector.tensor_scalar_mul(negm[:], srt[:, 0:1], -1.0)
                ex = sb.tile([128, 8], F32, name="ex", tag="ex", bufs=4)
                nc.scalar.activation(ex[:], lg[:], AF.Exp, bias=negm[:, 0:1],
                                     scale=1.0, accum_out=ssum[:, tc_i:tc_i + 1])
                exsel = sb.tile([128, 8], F32, name="exsel", tag="exsel", bufs=4)
                nc.vector.tensor_tensor(out=exsel[:], in0=ex[:], in1=esel_sb[:],
                                        op=ALU.mult)
                junk = sb.tile([128, 8], F32, name="junk", tag="junk", bufs=4)
                nc.vector.scalar_tensor_tensor(
                    out=junk[:], in0=lg[:], scalar=srt[:, 1:2], in1=exsel[:],
                    op0=ALU.is_ge, op1=ALU.mult,
                    accum_out=graw[:, tc_i:tc_i + 1])
        rcp = sb.tile([128, NT], F32)
        nc.vector.reciprocal(rcp[:], ssum[:])
        gall = sb.tile([128, NT], F32)   # gate of expert c per token (0 if off)
        nc.vector.tensor_tensor(out=gall[:], in0=graw[:], in1=rcp[:],
                                op=ALU.mult)

        # ---------------- compaction: pos[t] over token order ----------------
        # token t = f*128 + p  (tile f, partition p): prefix down columns.
        m16 = sb.tile([128, NT], F32)
        nc.vector.tensor_scalar(m16[:], gall[:], 0.0, None, op0=ALU.is_gt)
        pincl = psA.tile([128, NT], F32, name="pincl", tag="pA")
        nc.tensor.matmul(pincl[:], lhsT=uincl[:], rhs=m16[:],
                         start=True, stop=True)
        incl = sb.tile([128, NT], F32)
        nc.vector.tensor_copy(incl[:], pincl[:])
        # column totals = row 127 of incl, extracted via one-hot matmul
        selv = sb.tile([128, 1], F32)
        pv1 = sb.tile([128, 1], F32)
        nc.vector.tensor_copy(pv1[:], pvals_i[:, 0:1])
        nc.vector.tensor_scalar(selv[:], pv1[:], 127.0, None, op0=ALU.is_equal)
        pcolt = psA.tile([1, NT], F32, name="pcolt", tag="pA")
        nc.tensor.matmul(pcolt[:], lhsT=selv[:], rhs=incl[:],
                         start=True, stop=True)
        colt = sb.tile([1, NT], F32)
        nc.vector.tensor_copy(colt[:], pcolt[:])
        colp = sb.tile([1, NT], F32)
        nc.vector.tensor_copy(colp[:], colt[:])
        for sh in (1, 2, 4, 8):
            nc.vector.tensor_tensor(out=colp[:, sh:NT], in0=colp[:, sh:NT],
                                    in1=colp[:, 0:NT - sh], op=ALU.add)
        colex = sb.tile([1, NT], F32)   # exclusive prefix of column totals
        nc.vector.tensor_tensor(out=colex[:], in0=colp[:],
                                in1=colt[:], op=ALU.subtract)
        pbase = psA.tile([128, NT], F32, name="pbase", tag="pA")
        nc.tensor.matmul(pbase[:], lhsT=ones1[:], rhs=colex[:],
                         start=True, stop=True)
        posf = sb.tile([128, NT], F32)
        nc.vector.scalar_tensor_tensor(out=posf[:], in0=incl[:],
                                       scalar=BIG - 1.0, in1=pbase[:],
                                       op0=ALU.add, op1=ALU.add)
        bigm = sb.tile([128, NT], F32)
        nc.vector.tensor_scalar_mul(bigm[:], m16[:], BIG)
        nc.vector.tensor_tensor(out=posf[:], in0=posf[:], in1=bigm[:],
                                op=ALU.subtract)

        # per-dest-range rank (dest range r = f>>1 covers 256 tokens):
        # rankr[p,f] = incl[p,f] - 1 + (f odd ? coltot[f-1] : 0)
        ovrow = sb.tile([1, NT], F32)
        ovv = ovrow[:].rearrange("p (a b) -> p a b", b=2)
        nc.vector.memset(ovv[:, :, 0], 0.0)
        nc.vector.memset(ovv[:, :, 1], 1.0)
        fvrow = sb.tile([1, NT], F32)
        nc.vector.tensor_copy(fvrow[:], fvals_i[0:1, :])
        qvrow = sb.tile([1, NT], F32)   # f >> 1
        nc.vector.scalar_tensor_tensor(out=qvrow[:], in0=ovrow[:],
                                       scalar=-1.0, in1=fvrow[:],
                                       op0=ALU.mult, op1=ALU.add)
        nc.vector.tensor_scalar_mul(qvrow[:], qvrow[:], 0.5)
        colexr = sb.tile([1, NT], F32)
        nc.vector.memset(colexr[:, 0:1], 0.0)
        nc.vector.tensor_tensor(out=colexr[:, 1:NT], in0=ovrow[:, 1:NT],
                                in1=colt[:, 0:NT - 1], op=ALU.mult)
        prank = psA.tile([128, NT], F32, name="prank", tag="pA")
        nc.tensor.matmul(prank[:], lhsT=ones1[:], rhs=colexr[:],
                         start=True, stop=True)
        rankr = sb.tile([128, NT], F32)
        nc.vector.scalar_tensor_tensor(out=rankr[:], in0=incl[:],
                                       scalar=-1.0, in1=prank[:],
                                       op0=ALU.add, op1=ALU.add)
        pq = psA.tile([128, NT], F32, name="pq", tag="pA")
        nc.tensor.matmul(pq[:], lhsT=ones1[:], rhs=qvrow[:],
                         start=True, stop=True)

        # ---------------- PT one-hot (for extraction) ----------------
        PT = [sb.tile([128, CAP], BF16, name=f"PT{t}") for t in range(NT)]
        for t in range(NT):
            nc.vector.tensor_scalar(PT[t][:], iota640f[:], posf[:, t:t + 1],
                                    None, op0=ALU.is_equal)

        # ---------------- slot -> (token, gate, rank, dest) extraction ------
        # tg[p, f, :] = [f, p, gate, rankr, q] in bf16 (all exact in bf16)
        NR = 5
        tg = sb.tile([128, NT * NR], BF16)
        tgv = tg[:].rearrange("p (f a) -> p f a", a=NR)
        fv_b = sb.tile([128, NT], BF16)
        nc.vector.tensor_copy(fv_b[:], fvals_i[:])
        pv_b = sb.tile([128, NT], BF16)
        nc.vector.tensor_copy(pv_b[:], pvals_i[:])
        nc.vector.tensor_copy(tgv[:, :, 0], fv_b[:])
        nc.vector.tensor_copy(tgv[:, :, 1], pv_b[:])
        nc.vector.tensor_copy(tgv[:, :, 2], gall[:])
        nc.vector.tensor_copy(tgv[:, :, 3], rankr[:])
        nc.vector.tensor_copy(tgv[:, :, 4], pq[:])
        ext = sb.tile([NR, CAP], F32)
        for off, w in ((0, 512), (512, 128)):
            pe = psA.tile([NR, w], F32, name="pe", tag="pA")
            for t in range(NT):
                nc.tensor.matmul(pe[:], lhsT=tgv[:, t, :],
                                 rhs=PT[t][:, off:off + w],
                                 start=(t == 0), stop=(t == NT - 1))
            nc.vector.tensor_copy(ext[:, off:off + w], pe[:])
        idx_i = []
        a2a_i = []
        il_b = []
        g_sel = []
        for ct in range(CAP // 128):
            pext = psA.tile([128, NR], F32, name="pext", tag="pA")
            nc.tensor.transpose(pext[:], ext[:, ct * 128:(ct + 1) * 128],
                                ident[:NR, :NR])
            exr = sb.tile([128, NR], F32, name="exr", tag="exr", bufs=5)
            nc.vector.tensor_copy(exr[:], pext[:])
            gs = sb.tile([128, 1], F32, name=f"g_sel{ct}")
            nc.vector.tensor_copy(gs[:], exr[:, 2:3])
            g_sel.append(gs)
            vmask = sb.tile([128, 1], F32, name="vmask", tag="vmask", bufs=5)
            nc.vector.tensor_scalar(vmask[:], gs[:], 0.0, None, op0=ALU.is_le)
            # token idx = f*128 + p (for the x gather); +BIG on pad slots
            idxr = sb.tile([128, 1], F32, name="idxr", tag="idxr", bufs=5)
            nc.vector.scalar_tensor_tensor(
                out=idxr[:], in0=exr[:, 0:1], scalar=128.0, in1=exr[:, 1:2],
                op0=ALU.mult, op1=ALU.add)
            # idx within dest range = token - 256*q  (0..255, bf16-exact)
            ilf = sb.tile([128, 1], F32, name="ilf", tag="ilf", bufs=5)
            nc.vector.scalar_tensor_tensor(
                out=ilf[:], in0=exr[:, 4:5], scalar=-256.0, in1=idxr[:],
                op0=ALU.mult, op1=ALU.add)
            ilb = sb.tile([128, 1], BF16, name=f"il_b{ct}")
            nc.vector.tensor_copy(ilb[:], ilf[:])
            il_b.append(ilb)
            # a2a scatter row = 128*q + rankr; +BIG on pad slots
            prf = sb.tile([128, 1], F32, name="prf", tag="prf", bufs=5)
            nc.vector.scalar_tensor_tensor(
                out=prf[:], in0=exr[:, 4:5], scalar=float(CAPD),
                in1=exr[:, 3:4], op0=ALU.mult, op1=ALU.add)
            nc.vector.scalar_tensor_tensor(
                out=prf[:], in0=vmask[:], scalar=BIG, in1=prf[:],
                op0=ALU.mult, op1=ALU.add)
            ai = sb.tile([128, 1], I32, name=f"a2a_i{ct}")
            nc.vector.tensor_copy(ai[:], prf[:])
            a2a_i.append(ai)
            idxf = sb.tile([128, 1], F32, name="idxf", tag="idxf", bufs=5)
            nc.vector.scalar_tensor_tensor(
                out=idxf[:], in0=vmask[:], scalar=BIG, in1=idxr[:],
                op0=ALU.mult, op1=ALU.add)
            ii = sb.tile([128, 1], I32, name=f"idx_i{ct}")
            nc.vector.tensor_copy(ii[:], idxf[:])
            idx_i.append(ii)

        # ---------------- gather tokens (bf16) + transpose ----------------
        xTc = [sb.tile([128, CAP], BF16, name=f"xTc{d}") for d in range(DC)]
        for ct in range(CAP // 128):
            xg = sb.tile([128, D], BF16, name="xg", tag="xg", bufs=3)
            nc.vector.memset(xg[:], 0.0)
            nc.gpsimd.indirect_dma_start(
                out=xg[:], out_offset=None,
                in_=x2[:],
                in_offset=bass.IndirectOffsetOnAxis(ap=idx_i[ct][:, 0:1],
                                                    axis=0),
                bounds_check=N - 1, oob_is_err=False)
            for d in range(DC):
                ptx = psA.tile([128, 128], BF16, name="ptx", tag="pA")
                nc.tensor.transpose(ptx[:], xg[:, d * 128:(d + 1) * 128],
                                    identb[:])
                nc.vector.tensor_copy(xTc[d][:, ct * 128:(ct + 1) * 128],
                                      ptx[:])
        # warm the collective stream while stage 1/2 run
        nc.gpsimd.collective_compute(
            "AllGather", ALU.bypass, replica_groups=[CORE_IDS],
            ins=[warm_in.opt()], outs=[warm_out.opt()])

        # w2 loads (scalar queue, after routing's exp calls in program order)
        for h in range(HC):
            nc.scalar.dma_start(out=w2sb[h][:],
                                in_=w2[h * 128:(h + 1) * 128, :])

        # ---------------- stage 1: hT = gelu(w1^T xTc) (bf16) ----------------
        hT = [sb.tile([128, CAP], BF16, name=f"hT{h}") for h in range(HC)]
        for hc in range(HC):
            for off, w in ((0, 512), (512, 128)):
                ph = ps1.tile([128, w], F32, name="ph", tag="p1")
                for d in range(DC):
                    nc.tensor.matmul(
                        ph[:], lhsT=w1sb[d][:, hc * 128:(hc + 1) * 128],
                        rhs=xTc[d][:, off:off + w],
                        start=(d == 0), stop=(d == DC - 1))
                nc.scalar.activation(hT[hc][:, off:off + w], ph[:],
                                     AF.Gelu_apprx_tanh)

        # ---------------- stage 2 + gated scatter into a2a blocks ----------
        for ct in range(CAP // 128):
            y_sb = sb.tile([128, AW], BF16, name="y_sb", tag="y_sb", bufs=3)
            for half in range(2):
                py = ps2.tile([128, 384], F32, name="py", tag="p2")
                for hc in range(HC):
                    nc.tensor.matmul(
                        py[:], lhsT=hT[hc][:, ct * 128:(ct + 1) * 128],
                        rhs=w2sb[hc][:, half * 384:(half + 1) * 384],
                        start=(hc == 0), stop=(hc == HC - 1))
                nc.vector.tensor_scalar_mul(
                    y_sb[:, half * 384:(half + 1) * 384], py[:],
                    g_sel[ct][:, 0:1])
            nc.vector.tensor_copy(y_sb[:, D:D + 1], il_b[ct][:])
            nc.vector.memset(y_sb[:, D + 1:AW], 0.0)
            nc.gpsimd.indirect_dma_start(
                out=a2a_send[:],
                out_offset=bass.IndirectOffsetOnAxis(ap=a2a_i[ct][:, 0:1],
                                                     axis=0),
                in_=y_sb[:], in_offset=None,
                bounds_check=N_CORES * CAPD - 1, oob_is_err=False)

        # ---------------- combine: AllToAll + local matmul-sum -------------
        nc.gpsimd.collective_compute(
            "AllToAll", ALU.bypass, replica_groups=[CORE_IDS],
            ins=[a2a_send.opt()], outs=[a2a_recv.opt()])
        rcv = []
        pcb = []
        for sb_ in range(N_CORES):
            rt = sb.tile([128, AW], BF16, name=f"rcv{sb_}")
            nc.sync.dma_start(out=rt[:],
                              in_=a2a_recv[sb_ * CAPD:(sb_ + 1) * CAPD, :])
            rcv.append(rt)
            ixf = sb.tile([128, 1], F32, name="ixf", tag="ixf", bufs=4)
            nc.vector.tensor_copy(ixf[:], rt[:, D:D + 1])
            pc = sb.tile([128, NS], BF16, name=f"pcb{sb_}")
            nc.vector.tensor_scalar(pc[:], iota640f[:, 0:NS], ixf[:, 0:1],
                                    None, op0=ALU.is_equal)
            pcb.append(pc)
        for tc2 in range(NS // 128):
            fin = sb.tile([128, D], F32, name="fin", tag="fin", bufs=2)
            for half in range(2):
                po = ps2.tile([128, 384], F32, name="po", tag="p2")
                for sb_ in range(N_CORES):
                    nc.tensor.matmul(
                        po[:], lhsT=pcb[sb_][:, tc2 * 128:(tc2 + 1) * 128],
                        rhs=rcv[sb_][:, half * 384:(half + 1) * 384],
                        start=(sb_ == 0), stop=(sb_ == N_CORES - 1))
                nc.vector.tensor_copy(fin[:, half * 384:(half + 1) * 384],
                                      po[:])
            nc.sync.dma_start(out=out[tc2 * 128:(tc2 + 1) * 128, :],
                              in_=fin[:])

    nc.compile()
    return nc


_NC_CACHE = None


def _get_nc():
    global _NC_CACHE
    if _NC_CACHE is None:
        _NC_CACHE = build()
    return _NC_CACHE


def _make_in_maps(inp):
    inputs = np.ascontiguousarray(inp["inputs"], dtype=np.float32)
    router_w = np.ascontiguousarray(inp["router_w"], dtype=np.float32)
    w1 = np.asarray(inp["w1"], dtype=np.float32)
    w2 = np.asarray(inp["w2"], dtype=np.float32)
    B, S, Dm = inputs.shape
    xfull = inputs.reshape(-1, Dm)
    xbf = np.ascontiguousarray(xfull.astype(ml_dtypes.bfloat16))
    rwt = np.ascontiguousarray(router_w.T)
    w1b = np.ascontiguousarray(w1.astype(ml_dtypes.bfloat16))
    w2b = np.ascontiguousarray(w2.astype(ml_dtypes.bfloat16))
    in_maps = []
    for c in CORE_IDS:
        ese = np.zeros((128, E), dtype=np.float32)
        ese[:, c] = 1.0
        in_maps.append({
            "x": xfull,
            "x2": xbf,
            "rwt": rwt,
            "w1": w1b[c],
            "w2": w2b[c],
            "esel": ese,
        })
    return in_maps


def kernel(inputs, router_w, w1, w2, _run_kwargs=None):
    B, S, Dm = inputs.shape
    in_maps = _make_in_maps({"inputs": inputs, "router_w": router_w,
                             "w1": w1, "w2": w2})
    nc = _get_nc()
    res = run_bass_kernel_spmd(nc, in_maps, CORE_IDS, **(_run_kwargs or {}))
    shards = [res.results[c]["out"] for c in CORE_IDS]
    out = np.concatenate(shards, axis=0).reshape(B, S, Dm)
    if _run_kwargs:
        kernel.last_results = res
    return out


# revision 45
# speedup vs baseline: 1.0812x; 1.0812x over previous
"""Expert-parallel MoE (8 experts, top-2, D=768, H=3072, N=2048) on 8 trn2 cores.

v2 design (vs baseline):
- Routing is computed fully locally on every core (replicated) -- the mid-kernel
  AllGather and its barrier serialization are gone; the only collective is the
  final ReduceScatter, whose stream-init barrier overlaps local compute.
- MLP runs in bf16 (host-cast weights + on-chip x cast); ReduceScatter payload
  is bf16 (halves collective bytes).
- Token dispatch (compaction) is a one-hot matmul: PT[t, slot] = (pos[t]==slot)
  built with iota/is_eq; compact xT = sum_t x[t,:]^T PT[t,:]. No indirect-DMA
  scatter tables.  Slot->token index + gate are recovered with one small matmul
  chain against PT, so only the 5 y-row scatters use indirect DMA.
"""
import numpy as np
import ml_dtypes

import concourse.bass as bass
import concourse.tile as tile
import concourse.mybir as mybir
from concourse import bacc
from concourse.bass_utils import run_bass_kernel_spmd
from concourse.masks import make_identity, make_upper_triangular

F32 = mybir.dt.float32
F32R = mybir.dt.float32r
BF16 = mybir.dt.bfloat16
I32 = mybir.dt.int32
AF = mybir.ActivationFunctionType
ALU = mybir.AluOpType

N_CORES = 8
CORE_IDS = list(range(N_CORES))

N = 2048            # tokens
D = 768             # d_model
H = 3072            # d_ff
E = 8               # experts
NS = N // N_CORES   # output tokens per core (256)
CAP = 640           # per-expert token capacity (max observed load 557)
NT = N // 128       # 16 token tiles
DC = D // 128       # 6 d chunks
HC = H // 128       # 24 h chunks
BIG = float(1 << 20)


def build():
    nc = bacc.Bacc("TRN2", target_bir_lowering=False, debug=False,
                   num_devices=N_CORES)

    x = nc.dram_tensor("x", [N, D], F32, kind="ExternalInput").ap()
    x2 = nc.dram_tensor("x2", [N, D], BF16, kind="ExternalInput").ap()
    rwt = nc.dram_tensor("rwt", [D, E], F32, kind="ExternalInput").ap()
    w1 = nc.dram_tensor("w1", [D, H], BF16, kind="ExternalInput").ap()
    w2 = nc.dram_tensor("w2", [H, D], BF16, kind="ExternalInput").ap()
    esel = nc.dram_tensor("esel", [128, E], F32, kind="ExternalInput").ap()
    out = nc.dram_tensor("out", [NS, D], F32, kind="ExternalOutput").ap()

    from contextlib import ExitStack
    with tile.TileContext(nc) as tc, ExitStack() as ctx:
        sb = ctx.enter_context(tc.tile_pool(name="sb", bufs=1))
        psA = ctx.enter_context(tc.tile_pool(name="psA", bufs=3, space="PSUM"))
        ps1 = ctx.enter_context(tc.tile_pool(name="ps1", bufs=3, space="PSUM"))
        ps2 = ctx.enter_context(tc.tile_pool(name="ps2", bufs=2, space="PSUM"))
        xsp = ctx.enter_context(tc.tile_pool(name="xsp", bufs=4))
        xtp = ctx.enter_context(tc.tile_pool(name="xtp", bufs=1))
        dr = ctx.enter_context(tc.tile_pool(name="dr", bufs=1, space="DRAM"))

        # ---------------- DRAM scratch ----------------
        out_full = dr.tile([N, D], BF16)
        rs_out = dr.tile([NS, D], BF16)
        warm_in = dr.tile([8, 32], BF16)
        warm_out = dr.tile([64, 32], BF16, addr_space="Shared")

        # ---------------- constants ----------------
        ident = sb.tile([128, 128], F32)
        make_identity(nc, ident[:])
        identb = sb.tile([128, 128], BF16)
        make_identity(nc, identb[:])
        uincl = sb.tile([128, 128], F32)   # [q <= p] as lhsT: incl prefix
        make_upper_triangular(nc, uincl[:], val=1.0, diag=True)
        ones1 = sb.tile([1, 128], F32)
        nc.vector.memset(ones1[:], 1.0)
        iota640i = sb.tile([128, CAP], I32)
        nc.gpsimd.iota(iota640i[:], pattern=[[1, CAP]], base=0,
                       channel_multiplier=0)
        iota640f = sb.tile([128, CAP], F32)
        nc.vector.tensor_copy(iota640f[:], iota640i[:])
        fvals_i = sb.tile([128, NT], I32)   # col f -> f
        nc.gpsimd.iota(fvals_i[:], pattern=[[1, NT]], base=0,
                       channel_multiplier=0)
        pvals_i = sb.tile([128, NT], I32)   # value p everywhere
        nc.gpsimd.iota(pvals_i[:], pattern=[[0, NT]], base=0,
                       channel_multiplier=1)
        esel_sb = sb.tile([128, E], F32)
        nc.sync.dma_start(out=esel_sb[:], in_=esel[:])
        zero_row = sb.tile([128, D], BF16)
        nc.vector.memset(zero_row[:], 0.0)
        rwt_sb = [sb.tile([128, E], F32, name=f"rwt_sb{d}") for d in range(DC)]
        for d in range(DC):
            nc.scalar.dma_start(out=rwt_sb[d][:],
                                in_=rwt[d * 128:(d + 1) * 128, :])

        # ---------------- early bulk DMAs ----------------
        # x f32 (critical path head) + x bf16 on sync HW queue
        xs_t = [xsp.tile([128, D], F32, name="xs", tag="xs") for _ in range(NT)]
        for t in range(NT):
            nc.sync.dma_start(out=xs_t[t][:], in_=x[t * 128:(t + 1) * 128, :])
        # w1/w2 DMAs are issued later in the scalar program order (after
        # routing) so they don't compete with x for DMA engines at the head.
        w1sb = [sb.tile([128, H], BF16, name=f"w1sb{d}") for d in range(DC)]
        w2sb = [sb.tile([128, D], BF16, name=f"w2sb{h}") for h in range(HC)]
        # zero-init combine buffer early on the gpsimd queue (the dummy
        # stream-warming AllGather fires later, after the token gathers)
        nc.gpsimd.dma_start(out=warm_in[:], in_=zero_row[0:8, 0:32])
        for t in range(NT):
            nc.gpsimd.dma_start(out=out_full[t * 128:(t + 1) * 128, :],
                                in_=zero_row[:])

        # ---------------- routing (all 2048 tokens, f32) ----------------
        ssum = sb.tile([128, NT], F32)
        graw = sb.tile([128, NT], F32)
        for g in range(4):          # groups of 4 token tiles (512 tokens)
            xT = [xtp.tile([128, 512], F32, name="xT", tag=f"xT{d}")
                  for d in range(DC)]
            for d in range(DC):
                ptg = psA.tile([128, 512], F32, name="ptg", tag="pA")
                for t in range(4):
                    nc.tensor.transpose(
                        ptg[:, t * 128:(t + 1) * 128],
                        xs_t[4 * g + t][:, d * 128:(d + 1) * 128], ident[:])
                nc.vector.tensor_copy(xT[d][:], ptg[:])
            pl = psA.tile([8, 512], F32, name="pl", tag="pA")
            for d in range(DC):
                nc.tensor.matmul(pl[:], lhsT=rwt_sb[d][:], rhs=xT[d][:],
                                 start=(d == 0), stop=(d == DC - 1))
            l_sb = sb.tile([8, 512], F32, name="l_sb", tag="l_sb", bufs=2)
            nc.vector.tensor_copy(l_sb[:], pl[:])
            for t in range(4):
                tc_i = 4 * g + t
                ptl = psA.tile([128, 8], F32, name="ptl", tag="pA")
                nc.tensor.transpose(ptl[:], l_sb[:, t * 128:(t + 1) * 128],
                                    ident[:8, :8])
                lg = sb.tile([128, 8], F32, name="lg", tag="lg", bufs=4)
                nc.vector.tensor_copy(lg[:], ptl[:])
                srt = sb.tile([128, 8], F32, name="srt", tag="srt", bufs=4)
                nc.vector.max(srt[:], lg[:])
                negm = sb.tile([128, 1], F32, name="negm", tag="negm", bufs=4)
                nc.vector.tensor_scalar_mul(negm[:], srt[:, 0:1], -1.0)
                ex = sb.tile([128, 8], F32, name="ex", tag="ex", bufs=4)
                nc.scalar.activation(ex[:], lg[:], AF.Exp, bias=negm[:, 0:1],
                                     scale=1.0, accum_out=ssum[:, tc_i:tc_i + 1])
                exsel = sb.tile([128, 8], F32, name="exsel", tag="exsel", bufs=4)
                nc.vector.tensor_tensor(out=exsel[:], in0=ex[:], in1=esel_sb[:],
                                        op=ALU.mult)
                junk = sb.tile([128, 8], F32, name="junk", tag="junk", bufs=4)
                nc.vector.scalar_tensor_tensor(
                    out=junk[:], in0=lg[:], scalar=srt[:, 1:2], in1=exsel[:],
                    op0=ALU.is_ge, op1=ALU.mult,
                    accum_out=graw[:, tc_i:tc_i + 1])
        rcp = sb.tile([128, NT], F32)
        nc.vector.reciprocal(rcp[:], ssum[:])
        gall = sb.tile([128, NT], F32)   # gate of expert c per token (0 if off)
        nc.vector.tensor_tensor(out=gall[:], in0=graw[:], in1=rcp[:],
                                op=ALU.mult)

        # w1 then w2 loads (scalar queue, after routing in program order)
        for d in range(DC):
            nc.scalar.dma_start(out=w1sb[d][:],
                                in_=w1[d * 128:(d + 1) * 128, :])

        # ---------------- compaction: pos[t] over token order ----------------
        # token t = f*128 + p  (tile f, partition p): prefix down columns.
        m16 = sb.tile([128, NT], F32)
        nc.vector.tensor_scalar(m16[:], gall[:], 0.0, None, op0=ALU.is_gt)
        pincl = psA.tile([128, NT], F32, name="pincl", tag="pA")
        nc.tensor.matmul(pincl[:], lhsT=uincl[:], rhs=m16[:],
                         start=True, stop=True)
        incl = sb.tile([128, NT], F32)
        nc.vector.tensor_copy(incl[:], pincl[:])
        # column totals = row 127 of incl, extracted via one-hot matmul
        selv = sb.tile([128, 1], F32)
        pv1 = sb.tile([128, 1], F32)
        nc.vector.tensor_copy(pv1[:], pvals_i[:, 0:1])
        nc.vector.tensor_scalar(selv[:], pv1[:], 127.0, None, op0=ALU.is_equal)
        pcolt = psA.tile([1, NT], F32, name="pcolt", tag="pA")
        nc.tensor.matmul(pcolt[:], lhsT=selv[:], rhs=incl[:],
                         start=True, stop=True)
        colt = sb.tile([1, NT], F32)
        nc.vector.tensor_copy(colt[:], pcolt[:])
        colp = sb.tile([1, NT], F32)
        nc.vector.tensor_copy(colp[:], colt[:])
        for sh in (1, 2, 4, 8):
            nc.vector.tensor_tensor(out=colp[:, sh:NT], in0=colp[:, sh:NT],
                                    in1=colp[:, 0:NT - sh], op=ALU.add)
        colex = sb.tile([1, NT], F32)   # exclusive prefix of column totals
        nc.vector.tensor_tensor(out=colex[:], in0=colp[:],
                                in1=colt[:], op=ALU.subtract)
        pbase = psA.tile([128, NT], F32, name="pbase", tag="pA")
        nc.tensor.matmul(pbase[:], lhsT=ones1[:], rhs=colex[:],
                         start=True, stop=True)
        posf = sb.tile([128, NT], F32)
        nc.vector.scalar_tensor_tensor(out=posf[:], in0=incl[:],
                                       scalar=BIG - 1.0, in1=pbase[:],
                                       op0=ALU.add, op1=ALU.add)
        bigm = sb.tile([128, NT], F32)
        nc.vector.tensor_scalar_mul(bigm[:], m16[:], BIG)
        nc.vector.tensor_tensor(out=posf[:], in0=posf[:], in1=bigm[:],
                                op=ALU.subtract)

        # ---------------- PT one-hot (for extraction) ----------------
        PT = [sb.tile([128, CAP], BF16, name=f"PT{t}") for t in range(NT)]
        for t in range(NT):
            nc.vector.tensor_scalar(PT[t][:], iota640f[:], posf[:, t:t + 1],
                                    None, op0=ALU.is_equal)

        # ---------------- slot -> (token, gate, valid) extraction -----------
        # tg[p, f, :] = [f, p, gate, 1] in bf16 (all exactly representable)
        NR = 4
        tg = sb.tile([128, NT * NR], BF16)
        tgv = tg[:].rearrange("p (f a) -> p f a", a=NR)
        fv_b = sb.tile([128, NT], BF16)
        nc.vector.tensor_copy(fv_b[:], fvals_i[:])
        pv_b = sb.tile([128, NT], BF16)
        nc.vector.tensor_copy(pv_b[:], pvals_i[:])
        nc.vector.tensor_copy(tgv[:, :, 0], fv_b[:])
        nc.vector.tensor_copy(tgv[:, :, 1], pv_b[:])
        nc.vector.tensor_copy(tgv[:, :, 2], gall[:])
        nc.vector.memset(tgv[:, :, 3], 1.0)
        ext = sb.tile([NR, CAP], F32)
        for off, w in ((0, 512), (512, 128)):
            pe = psA.tile([NR, w], F32, name="pe", tag="pA")
            for t in range(NT):
                nc.tensor.matmul(pe[:], lhsT=tgv[:, t, :],
                                 rhs=PT[t][:, off:off + w],
                                 start=(t == 0), stop=(t == NT - 1))
            nc.vector.tensor_copy(ext[:, off:off + w], pe[:])
        idx_i = []
        g_sel = []
        for ct in range(CAP // 128):
            pext = psA.tile([128, NR], F32, name="pext", tag="pA")
            nc.tensor.transpose(pext[:], ext[:, ct * 128:(ct + 1) * 128],
                                ident[:NR, :NR])
            exr = sb.tile([128, NR], F32, name="exr", tag="exr", bufs=5)
            nc.vector.tensor_copy(exr[:], pext[:])
            # token idx = f*128 + p + BIG*(1-valid); pads dropped via bounds
            idxf = sb.tile([128, 1], F32, name="idxf", tag="idxf", bufs=5)
            nc.vector.scalar_tensor_tensor(
                out=idxf[:], in0=exr[:, 0:1], scalar=128.0, in1=exr[:, 1:2],
                op0=ALU.mult, op1=ALU.add)
            nc.vector.scalar_tensor_tensor(
                out=idxf[:], in0=exr[:, 3:4], scalar=-BIG, in1=idxf[:],
                op0=ALU.mult, op1=ALU.add)
            nc.vector.tensor_scalar_add(idxf[:], idxf[:], BIG)
            ii = sb.tile([128, 1], I32, name=f"idx_i{ct}")
            nc.vector.tensor_copy(ii[:], idxf[:])
            idx_i.append(ii)
            gs = sb.tile([128, 1], F32, name=f"g_sel{ct}")
            nc.vector.tensor_copy(gs[:], exr[:, 2:3])
            g_sel.append(gs)

        # ---------------- gather tokens (bf16) + transpose ----------------
        xTc = [sb.tile([128, CAP], BF16, name=f"xTc{d}") for d in range(DC)]
        for ct in range(CAP // 128):
            xg = sb.tile([128, D], BF16, name="xg", tag="xg", bufs=3)
            nc.vector.memset(xg[:], 0.0)
            nc.gpsimd.indirect_dma_start(
                out=xg[:], out_offset=None,
                in_=x2[:],
                in_offset=bass.IndirectOffsetOnAxis(ap=idx_i[ct][:, 0:1],
                                                    axis=0),
                bounds_check=N - 1, oob_is_err=False)
            for d in range(DC):
                ptx = psA.tile([128, 128], BF16, name="ptx", tag="pA")
                nc.tensor.transpose(ptx[:], xg[:, d * 128:(d + 1) * 128],
                                    identb[:])
                nc.vector.tensor_copy(xTc[d][:, ct * 128:(ct + 1) * 128],
                                      ptx[:])
        # warm the collective stream while stage 1/2 run
        nc.gpsimd.collective_compute(
            "AllGather", ALU.bypass, replica_groups=[CORE_IDS],
            ins=[warm_in.opt()], outs=[warm_out.opt()])

        # w2 loads (scalar queue, after routing's exp calls in program order)
        for h in range(HC):
            nc.scalar.dma_start(out=w2sb[h][:],
                                in_=w2[h * 128:(h + 1) * 128, :])

        # ---------------- stage 1: hT = gelu(w1^T xTc) (bf16) ----------------
        hT = [sb.tile([128, CAP], BF16, name=f"hT{h}") for h in range(HC)]
        for hc in range(HC):
            for off, w in ((0, 512), (512, 128)):
                ph = ps1.tile([128, w], F32, name="ph", tag="p1")
                for d in range(DC):
                    nc.tensor.matmul(
                        ph[:], lhsT=w1sb[d][:, hc * 128:(hc + 1) * 128],
                        rhs=xTc[d][:, off:off + w],
                        start=(d == 0), stop=(d == DC - 1))
                nc.scalar.activation(hT[hc][:, off:off + w], ph[:],
                                     AF.Gelu_apprx_tanh)

        # ---------------- stage 2 + gated scatter ----------------
        for ct in range(CAP // 128):
            y_sb = sb.tile([128, D], BF16, name="y_sb", tag="y_sb", bufs=3)
            for half in range(2):
                py = ps2.tile([128, 384], F32, name="py", tag="p2")
                for hc in range(HC):
                    nc.tensor.matmul(
                        py[:], lhsT=hT[hc][:, ct * 128:(ct + 1) * 128],
                        rhs=w2sb[hc][:, half * 384:(half + 1) * 384],
                        start=(hc == 0), stop=(hc == HC - 1))
                nc.vector.tensor_scalar_mul(
                    y_sb[:, half * 384:(half + 1) * 384], py[:],
                    g_sel[ct][:, 0:1])
            nc.gpsimd.indirect_dma_start(
                out=out_full[:],
                out_offset=bass.IndirectOffsetOnAxis(ap=idx_i[ct][:, 0:1],
                                                     axis=0),
                in_=y_sb[:], in_offset=None,
                bounds_check=N - 1, oob_is_err=False)

        # ---------------- combine: bf16 ReduceScatter ----------------
        nc.gpsimd.collective_compute(
            "ReduceScatter", ALU.add, replica_groups=[CORE_IDS],
            ins=[out_full.opt()], outs=[rs_out.opt()])
        for t in range(NS // 128):
            fin = sb.tile([128, D], F32, name="fin", tag="fin", bufs=2)
            nc.gpsimd.dma_start(out=fin[:],
                                in_=rs_out[t * 128:(t + 1) * 128, :])
            nc.sync.dma_start(out=out[t * 128:(t + 1) * 128, :], in_=fin[:])

    nc.compile()
    return nc


_NC_CACHE = None


def _get_nc():
    global _NC_CACHE
    if _NC_CACHE is None:
        _NC_CACHE = build()
    return _NC_CACHE


def _make_in_maps(inp):
    inputs = np.ascontiguousarray(inp["inputs"], dtype=np.float32)
    router_w = np.ascontiguousarray(inp["router_w"], dtype=np.float32)
    w1 = np.asarray(inp["w1"], dtype=np.float32)
    w2 = np.asarray(inp["w2"], dtype=np.float32)
    B, S, Dm = inputs.shape
    xfull = inputs.reshape(-1, Dm)
    xbf = np.ascontiguousarray(xfull.astype(ml_dtypes.bfloat16))
    rwt = np.ascontiguousarray(router_w.T)
    w1b = np.ascontiguousarray(w1.astype(ml_dtypes.bfloat16))
    w2b = np.ascontiguousarray(w2.astype(ml_dtypes.bfloat16))
    in_maps = []
    for c in CORE_IDS:
        ese = np.zeros((128, E), dtype=np.float32)
        ese[:, c] = 1.0
        in_maps.append({
            "x": xfull,
            "x2": xbf,
            "rwt": rwt,
            "w1": w1b[c],
            "w2": w2b[c],
            "esel": ese,
        })
    return in_maps


def kernel(inputs, router_w, w1, w2, _run_kwargs=None):
    B, S, Dm = inputs.shape
    in_maps = _make_in_maps({"inputs": inputs, "router_w": router_w,
                             "w1": w1, "w2": w2})
    nc = _get_nc()
    res = run_bass_kernel_spmd(nc, in_maps, CORE_IDS, **(_run_kwargs or {}))
    shards = [res.results[c]["out"] for c in CORE_IDS]
    out = np.concatenate(shards, axis=0).reshape(B, S, Dm)
    if _run_kwargs:
        kernel.last_results = res
    return out


# revision 53
# speedup vs baseline: 1.1039x; 1.0210x over previous
"""Expert-parallel MoE (8 experts, top-2, D=768, H=3072, N=2048) on 8 trn2 cores.

v2 design (vs baseline):
- Routing is computed fully locally on every core (replicated) -- the mid-kernel
  AllGather and its barrier serialization are gone; the only collective is the
  final ReduceScatter, whose stream-init barrier overlaps local compute.
- MLP runs in bf16 (host-cast weights + on-chip x cast); ReduceScatter payload
  is bf16 (halves collective bytes).
- Token dispatch (compaction) is a one-hot matmul: PT[t, slot] = (pos[t]==slot)
  built with iota/is_eq; compact xT = sum_t x[t,:]^T PT[t,:]. No indirect-DMA
  scatter tables.  Slot->token index + gate are recovered with one small matmul
  chain against PT, so only the 5 y-row scatters use indirect DMA.
"""
import numpy as np
import ml_dtypes

import concourse.bass as bass
import concourse.tile as tile
import concourse.mybir as mybir
from concourse import bacc
from concourse.bass_utils import run_bass_kernel_spmd
from concourse.masks import make_identity, make_upper_triangular

F32 = mybir.dt.float32
F32R = mybir.dt.float32r
BF16 = mybir.dt.bfloat16
I32 = mybir.dt.int32
AF = mybir.ActivationFunctionType
ALU = mybir.AluOpType

N_CORES = 8
CORE_IDS = list(range(N_CORES))

N = 2048            # tokens
D = 768             # d_model
H = 3072            # d_ff
E = 8               # experts
NS = N // N_CORES   # output tokens per core (256)
CAP = 640           # per-expert token capacity (max observed load 557)
NT = N // 128       # 16 token tiles
DC = D // 128       # 6 d chunks
HC = H // 128       # 24 h chunks
BIG = float(1 << 20)


def build():
    nc = bacc.Bacc("TRN2", target_bir_lowering=False, debug=False,
                   num_devices=N_CORES)

    x = nc.dram_tensor("x", [N, D], F32, kind="ExternalInput").ap()
    x2 = nc.dram_tensor("x2", [N, D], BF16, kind="ExternalInput").ap()
    rwt = nc.dram_tensor("rwt", [D, E], F32, kind="ExternalInput").ap()
    w1 = nc.dram_tensor("w1", [D, H], BF16, kind="ExternalInput").ap()
    w2 = nc.dram_tensor("w2", [H, D], BF16, kind="ExternalInput").ap()
    esel = nc.dram_tensor("esel", [128, E], F32, kind="ExternalInput").ap()
    out = nc.dram_tensor("out", [NS, D], F32, kind="ExternalOutput").ap()

    from contextlib import ExitStack
    with tile.TileContext(nc) as tc, ExitStack() as ctx:
        sb = ctx.enter_context(tc.tile_pool(name="sb", bufs=1))
        psA = ctx.enter_context(tc.tile_pool(name="psA", bufs=3, space="PSUM"))
        ps1 = ctx.enter_context(tc.tile_pool(name="ps1", bufs=3, space="PSUM"))
        ps2 = ctx.enter_context(tc.tile_pool(name="ps2", bufs=2, space="PSUM"))
        xsp = ctx.enter_context(tc.tile_pool(name="xsp", bufs=4))
        xtp = ctx.enter_context(tc.tile_pool(name="xtp", bufs=1))
        dr = ctx.enter_context(tc.tile_pool(name="dr", bufs=1, space="DRAM"))

        # ---------------- DRAM scratch ----------------
        # Combine buffer split at token 1024: slots are token-ordered, so the
        # low half is final after slot tile 2 and its ReduceScatter overlaps
        # stage 2 of tiles 3-4.  Each core ends up with tokens [128c,128c+128)
        # and [1024+128c, 1024+128c+128); the host reassembles.
        out_lo = dr.tile([N // 2, D], BF16)
        out_hi = dr.tile([N // 2, D], BF16)
        rs_lo = dr.tile([NS // 2, D], BF16)
        rs_hi = dr.tile([NS // 2, D], BF16)
        warm_in = dr.tile([8, 32], BF16)
        warm_out = dr.tile([64, 32], BF16, addr_space="Shared")

        # ---------------- constants ----------------
        ident = sb.tile([128, 128], F32)
        make_identity(nc, ident[:])
        identb = sb.tile([128, 128], BF16)
        make_identity(nc, identb[:])
        uincl = sb.tile([128, 128], F32)   # [q <= p] as lhsT: incl prefix
        make_upper_triangular(nc, uincl[:], val=1.0, diag=True)
        ones1 = sb.tile([1, 128], F32)
        nc.vector.memset(ones1[:], 1.0)
        iota640i = sb.tile([128, CAP], I32)
        nc.gpsimd.iota(iota640i[:], pattern=[[1, CAP]], base=0,
                       channel_multiplier=0)
        iota640f = sb.tile([128, CAP], F32)
        nc.vector.tensor_copy(iota640f[:], iota640i[:])
        fvals_i = sb.tile([128, NT], I32)   # col f -> f
        nc.gpsimd.iota(fvals_i[:], pattern=[[1, NT]], base=0,
                       channel_multiplier=0)
        pvals_i = sb.tile([128, NT], I32)   # value p everywhere
        nc.gpsimd.iota(pvals_i[:], pattern=[[0, NT]], base=0,
                       channel_multiplier=1)
        esel_sb = sb.tile([128, E], F32)
        nc.sync.dma_start(out=esel_sb[:], in_=esel[:])
        zero_row = sb.tile([128, D], BF16)
        nc.vector.memset(zero_row[:], 0.0)
        rwt_sb = [sb.tile([128, E], F32, name=f"rwt_sb{d}") for d in range(DC)]
        for d in range(DC):
            nc.scalar.dma_start(out=rwt_sb[d][:],
                                in_=rwt[d * 128:(d + 1) * 128, :])

        # ---------------- early bulk DMAs ----------------
        # x f32 (critical path head) + x bf16 on sync HW queue
        xs_t = [xsp.tile([128, D], F32, name="xs", tag="xs") for _ in range(NT)]
        for t in range(NT):
            nc.sync.dma_start(out=xs_t[t][:], in_=x[t * 128:(t + 1) * 128, :])
        # w1/w2 DMAs are issued later in the scalar program order (after
        # routing) so they don't compete with x for DMA engines at the head.
        w1sb = [sb.tile([128, H], BF16, name=f"w1sb{d}") for d in range(DC)]
        w2sb = [sb.tile([128, D], BF16, name=f"w2sb{h}") for h in range(HC)]
        # zero-init combine buffer early on the gpsimd queue (the dummy
        # stream-warming AllGather fires later, after the token gathers)
        nc.gpsimd.dma_start(out=warm_in[:], in_=zero_row[0:8, 0:32])
        for t in range(NT // 2):
            nc.gpsimd.dma_start(out=out_lo[t * 128:(t + 1) * 128, :],
                                in_=zero_row[:])
            nc.gpsimd.dma_start(out=out_hi[t * 128:(t + 1) * 128, :],
                                in_=zero_row[:])

        # ---------------- routing (all 2048 tokens, f32) ----------------
        ssum = sb.tile([128, NT], F32)
        graw = sb.tile([128, NT], F32)
        for g in range(4):          # groups of 4 token tiles (512 tokens)
            xT = [xtp.tile([128, 512], F32, name="xT", tag=f"xT{d}")
                  for d in range(DC)]
            for d in range(DC):
                ptg = psA.tile([128, 512], F32, name="ptg", tag="pA")
                for t in range(4):
                    nc.tensor.transpose(
                        ptg[:, t * 128:(t + 1) * 128],
                        xs_t[4 * g + t][:, d * 128:(d + 1) * 128], ident[:])
                nc.scalar.activation(xT[d][:], ptg[:], AF.Copy)
            pl = psA.tile([8, 512], F32, name="pl", tag="pA")
            for d in range(DC):
                nc.tensor.matmul(pl[:], lhsT=rwt_sb[d][:], rhs=xT[d][:],
                                 start=(d == 0), stop=(d == DC - 1))
            l_sb = sb.tile([8, 512], F32, name="l_sb", tag="l_sb", bufs=2)
            nc.scalar.activation(l_sb[:], pl[:], AF.Copy)
            for t in range(4):
                tc_i = 4 * g + t
                ptl = psA.tile([128, 8], F32, name="ptl", tag="pA")
                nc.tensor.transpose(ptl[:], l_sb[:, t * 128:(t + 1) * 128],
                                    ident[:8, :8])
                lg = sb.tile([128, 8], F32, name="lg", tag="lg", bufs=4)
                nc.vector.tensor_copy(lg[:], ptl[:])
                srt = sb.tile([128, 8], F32, name="srt", tag="srt", bufs=4)
                nc.vector.max(srt[:], lg[:])
                negm = sb.tile([128, 1], F32, name="negm", tag="negm", bufs=4)
                nc.vector.tensor_scalar_mul(negm[:], srt[:, 0:1], -1.0)
                ex = sb.tile([128, 8], F32, name="ex", tag="ex", bufs=4)
                nc.scalar.activation(ex[:], lg[:], AF.Exp, bias=negm[:, 0:1],
                                     scale=1.0, accum_out=ssum[:, tc_i:tc_i + 1])
                exsel = sb.tile([128, 8], F32, name="exsel", tag="exsel", bufs=4)
                nc.vector.tensor_tensor(out=exsel[:], in0=ex[:], in1=esel_sb[:],
                                        op=ALU.mult)
                junk = sb.tile([128, 8], F32, name="junk", tag="junk", bufs=4)
                nc.vector.scalar_tensor_tensor(
                    out=junk[:], in0=lg[:], scalar=srt[:, 1:2], in1=exsel[:],
                    op0=ALU.is_ge, op1=ALU.mult,
                    accum_out=graw[:, tc_i:tc_i + 1])
        rcp = sb.tile([128, NT], F32)
        nc.vector.reciprocal(rcp[:], ssum[:])
        gall = sb.tile([128, NT], F32)   # gate of expert c per token (0 if off)
        nc.vector.tensor_tensor(out=gall[:], in0=graw[:], in1=rcp[:],
                                op=ALU.mult)

        # w1 then w2 loads (scalar queue, after routing in program order)
        for d in range(DC):
            nc.scalar.dma_start(out=w1sb[d][:],
                                in_=w1[d * 128:(d + 1) * 128, :])

        # ---------------- compaction: pos[t] over token order ----------------
        # token t = f*128 + p  (tile f, partition p): prefix down columns.
        m16 = sb.tile([128, NT], F32)
        nc.vector.tensor_scalar(m16[:], gall[:], 0.0, None, op0=ALU.is_gt)
        pincl = psA.tile([128, NT], F32, name="pincl", tag="pA")
        nc.tensor.matmul(pincl[:], lhsT=uincl[:], rhs=m16[:],
                         start=True, stop=True)
        incl = sb.tile([128, NT], F32)
        nc.vector.tensor_copy(incl[:], pincl[:])
        # column totals = row 127 of incl, extracted via one-hot matmul
        selv = sb.tile([128, 1], F32)
        pv1 = sb.tile([128, 1], F32)
        nc.vector.tensor_copy(pv1[:], pvals_i[:, 0:1])
        nc.vector.tensor_scalar(selv[:], pv1[:], 127.0, None, op0=ALU.is_equal)
        pcolt = psA.tile([1, NT], F32, name="pcolt", tag="pA")
        nc.tensor.matmul(pcolt[:], lhsT=selv[:], rhs=incl[:],
                         start=True, stop=True)
        colt = sb.tile([1, NT], F32)
        nc.vector.tensor_copy(colt[:], pcolt[:])
        colp = sb.tile([1, NT], F32)
        nc.vector.tensor_copy(colp[:], colt[:])
        for sh in (1, 2, 4, 8):
            nc.vector.tensor_tensor(out=colp[:, sh:NT], in0=colp[:, sh:NT],
                                    in1=colp[:, 0:NT - sh], op=ALU.add)
        colex = sb.tile([1, NT], F32)   # exclusive prefix of column totals
        nc.vector.tensor_tensor(out=colex[:], in0=colp[:],
                                in1=colt[:], op=ALU.subtract)
        pbase = psA.tile([128, NT], F32, name="pbase", tag="pA")
        nc.tensor.matmul(pbase[:], lhsT=ones1[:], rhs=colex[:],
                         start=True, stop=True)
        posf = sb.tile([128, NT], F32)
        nc.vector.scalar_tensor_tensor(out=posf[:], in0=incl[:],
                                       scalar=BIG - 1.0, in1=pbase[:],
                                       op0=ALU.add, op1=ALU.add)
        bigm = sb.tile([128, NT], F32)
        nc.vector.tensor_scalar_mul(bigm[:], m16[:], BIG)
        nc.vector.tensor_tensor(out=posf[:], in0=posf[:], in1=bigm[:],
                                op=ALU.subtract)

        # ---------------- PT one-hot (for extraction) ----------------
        PT = [sb.tile([128, CAP], BF16, name=f"PT{t}") for t in range(NT)]
        for t in range(NT):
            nc.vector.tensor_scalar(PT[t][:], iota640f[:], posf[:, t:t + 1],
                                    None, op0=ALU.is_equal)

        # ---------------- slot -> (token, gate, valid) extraction -----------
        # tg[p, f, :] = [f, p, gate, 1] in bf16 (all exactly representable)
        NR = 4
        tg = sb.tile([128, NT * NR], BF16)
        tgv = tg[:].rearrange("p (f a) -> p f a", a=NR)
        fv_b = sb.tile([128, NT], BF16)
        nc.vector.tensor_copy(fv_b[:], fvals_i[:])
        pv_b = sb.tile([128, NT], BF16)
        nc.vector.tensor_copy(pv_b[:], pvals_i[:])
        nc.vector.tensor_copy(tgv[:, :, 0], fv_b[:])
        nc.vector.tensor_copy(tgv[:, :, 1], pv_b[:])
        nc.vector.tensor_copy(tgv[:, :, 2], gall[:])
        nc.vector.memset(tgv[:, :, 3], 1.0)
        ext = sb.tile([NR, CAP], F32)
        for off, w in ((0, 512), (512, 128)):
            pe = psA.tile([NR, w], F32, name="pe", tag="pA")
            for t in range(NT):
                nc.tensor.matmul(pe[:], lhsT=tgv[:, t, :],
                                 rhs=PT[t][:, off:off + w],
                                 start=(t == 0), stop=(t == NT - 1))
            nc.vector.tensor_copy(ext[:, off:off + w], pe[:])
        idx_i = []
        idx_f = []
        g_sel = []
        for ct in range(CAP // 128):
            pext = psA.tile([128, NR], F32, name="pext", tag="pA")
            nc.tensor.transpose(pext[:], ext[:, ct * 128:(ct + 1) * 128],
                                ident[:NR, :NR])
            exr = sb.tile([128, NR], F32, name="exr", tag="exr", bufs=5)
            nc.vector.tensor_copy(exr[:], pext[:])
            # token idx = f*128 + p + BIG*(1-valid); pads dropped via bounds
            idxf = sb.tile([128, 1], F32, name=f"idxf{ct}")
            nc.vector.scalar_tensor_tensor(
                out=idxf[:], in0=exr[:, 0:1], scalar=128.0, in1=exr[:, 1:2],
                op0=ALU.mult, op1=ALU.add)
            nc.vector.scalar_tensor_tensor(
                out=idxf[:], in0=exr[:, 3:4], scalar=-BIG, in1=idxf[:],
                op0=ALU.mult, op1=ALU.add)
            nc.vector.tensor_scalar_add(idxf[:], idxf[:], BIG)
            ii = sb.tile([128, 1], I32, name=f"idx_i{ct}")
            nc.vector.tensor_copy(ii[:], idxf[:])
            idx_i.append(ii)
            idx_f.append(idxf)
            gs = sb.tile([128, 1], F32, name=f"g_sel{ct}")
            nc.vector.tensor_copy(gs[:], exr[:, 2:3])
            g_sel.append(gs)

        # ---------------- gather tokens (bf16) + transpose ----------------
        xTc = [sb.tile([128, CAP], BF16, name=f"xTc{d}") for d in range(DC)]
        for ct in range(CAP // 128):
            xg = sb.tile([128, D], BF16, name="xg", tag="xg", bufs=3)
            nc.vector.memset(xg[:], 0.0)
            nc.gpsimd.indirect_dma_start(
                out=xg[:], out_offset=None,
                in_=x2[:],
                in_offset=bass.IndirectOffsetOnAxis(ap=idx_i[ct][:, 0:1],
                                                    axis=0),
                bounds_check=N - 1, oob_is_err=False)
            for d in range(DC):
                ptx = psA.tile([128, 128], BF16, name="ptx", tag="pA")
                nc.tensor.transpose(ptx[:], xg[:, d * 128:(d + 1) * 128],
                                    identb[:])
                nc.vector.tensor_copy(xTc[d][:, ct * 128:(ct + 1) * 128],
                                      ptx[:])
        # warm the collective stream while stage 1/2 run
        nc.gpsimd.collective_compute(
            "AllGather", ALU.bypass, replica_groups=[CORE_IDS],
            ins=[warm_in.opt()], outs=[warm_out.opt()])

        # w2 loads (scalar queue, after routing's exp calls in program order)
        for h in range(HC):
            nc.scalar.dma_start(out=w2sb[h][:],
                                in_=w2[h * 128:(h + 1) * 128, :])

        # ---------------- stage 1: hT = gelu(w1^T xTc) (bf16) ----------------
        hT = [sb.tile([128, CAP], BF16, name=f"hT{h}") for h in range(HC)]
        for hc in range(HC):
            for off, w in ((0, 512), (512, 128)):
                ph = ps1.tile([128, w], F32, name="ph", tag="p1")
                for d in range(DC):
                    nc.tensor.matmul(
                        ph[:], lhsT=w1sb[d][:, hc * 128:(hc + 1) * 128],
                        rhs=xTc[d][:, off:off + w],
                        start=(d == 0), stop=(d == DC - 1))
                nc.scalar.activation(hT[hc][:, off:off + w], ph[:],
                                     AF.Gelu_apprx_tanh)

        # ---------------- stage 2 + gated scatter ----------------
        # Tile ct's scatter targets: 0 -> lo only; 1,2 -> lo and hi (straddle
        # the token-1024 boundary); 3,4 -> hi only.  idx_hi = idx - 1024 with
        # +BIG protection against negative (lo-token) rows on tiles 1-2.
        idx_hi = []
        for ct in range(CAP // 128):
            if ct >= 1:
                ih = sb.tile([128, 1], F32, name="ih", tag="ih", bufs=4)
                mlo = sb.tile([128, 1], F32, name="mlo", tag="mlo", bufs=4)
                nc.vector.tensor_scalar(mlo[:], idx_f[ct][:], float(N // 2),
                                        None, op0=ALU.is_lt)
                nc.vector.tensor_scalar_add(ih[:], idx_f[ct][:],
                                            -float(N // 2))
                nc.vector.scalar_tensor_tensor(
                    out=ih[:], in0=mlo[:], scalar=BIG, in1=ih[:],
                    op0=ALU.mult, op1=ALU.add)
                ihi = sb.tile([128, 1], I32, name=f"idx_hi{ct}")
                nc.vector.tensor_copy(ihi[:], ih[:])
                idx_hi.append(ihi)
            else:
                idx_hi.append(None)
        for ct in range(CAP // 128):
            y_sb = sb.tile([128, D], BF16, name="y_sb", tag="y_sb", bufs=3)
            for half in range(2):
                py = ps2.tile([128, 384], F32, name="py", tag="p2")
                for hc in range(HC):
                    nc.tensor.matmul(
                        py[:], lhsT=hT[hc][:, ct * 128:(ct + 1) * 128],
                        rhs=w2sb[hc][:, half * 384:(half + 1) * 384],
                        start=(hc == 0), stop=(hc == HC - 1))
                nc.vector.tensor_scalar_mul(
                    y_sb[:, half * 384:(half + 1) * 384], py[:],
                    g_sel[ct][:, 0:1])
            if ct <= 2:
                nc.gpsimd.indirect_dma_start(
                    out=out_lo[:],
                    out_offset=bass.IndirectOffsetOnAxis(ap=idx_i[ct][:, 0:1],
                                                         axis=0),
                    in_=y_sb[:], in_offset=None,
                    bounds_check=N // 2 - 1, oob_is_err=False)
            if 1 <= ct <= 2:
                nc.gpsimd.indirect_dma_start(
                    out=out_hi[:],
                    out_offset=bass.IndirectOffsetOnAxis(
                        ap=idx_hi[ct][:, 0:1], axis=0),
                    in_=y_sb[:], in_offset=None,
                    bounds_check=N // 2 - 1, oob_is_err=False)
            if ct == 2:
                # low half is final: overlap its RS with tiles 3-4
                nc.gpsimd.collective_compute(
                    "ReduceScatter", ALU.add, replica_groups=[CORE_IDS],
                    ins=[out_lo.opt()], outs=[rs_lo.opt()])
            if ct >= 3:
                nc.gpsimd.indirect_dma_start(
                    out=out_hi[:],
                    out_offset=bass.IndirectOffsetOnAxis(
                        ap=idx_hi[ct][:, 0:1], axis=0),
                    in_=y_sb[:], in_offset=None,
                    bounds_check=N // 2 - 1, oob_is_err=False)
        nc.gpsimd.collective_compute(
            "ReduceScatter", ALU.add, replica_groups=[CORE_IDS],
            ins=[out_hi.opt()], outs=[rs_hi.opt()])
        fin = sb.tile([128, D], F32, name="fin", tag="fin", bufs=2)
        nc.gpsimd.dma_start(out=fin[:], in_=rs_lo[:])
        nc.sync.dma_start(out=out[0:128, :], in_=fin[:])
        fin2 = sb.tile([128, D], F32, name="fin2")
        nc.gpsimd.dma_start(out=fin2[:], in_=rs_hi[:])
        nc.sync.dma_start(out=out[128:256, :], in_=fin2[:])

    nc.compile()
    return nc


_NC_CACHE = None


def _get_nc():
    global _NC_CACHE
    if _NC_CACHE is None:
        _NC_CACHE = build()
    return _NC_CACHE


def _make_in_maps(inp):
    inputs = np.ascontiguousarray(inp["inputs"], dtype=np.float32)
    router_w = np.ascontiguousarray(inp["router_w"], dtype=np.float32)
    w1 = np.asarray(inp["w1"], dtype=np.float32)
    w2 = np.asarray(inp["w2"], dtype=np.float32)
    B, S, Dm = inputs.shape
    xfull = inputs.reshape(-1, Dm)
    xbf = np.ascontiguousarray(xfull.astype(ml_dtypes.bfloat16))
    rwt = np.ascontiguousarray(router_w.T)
    w1b = np.ascontiguousarray(w1.astype(ml_dtypes.bfloat16))
    w2b = np.ascontiguousarray(w2.astype(ml_dtypes.bfloat16))
    in_maps = []
    for c in CORE_IDS:
        ese = np.zeros((128, E), dtype=np.float32)
        ese[:, c] = 1.0
        in_maps.append({
            "x": xfull,
            "x2": xbf,
            "rwt": rwt,
            "w1": w1b[c],
            "w2": w2b[c],
            "esel": ese,
        })
    return in_maps


def kernel(inputs, router_w, w1, w2, _run_kwargs=None):
    B, S, Dm = inputs.shape
    in_maps = _make_in_maps({"inputs": inputs, "router_w": router_w,
                             "w1": w1, "w2": w2})
    nc = _get_nc()
    res = run_bass_kernel_spmd(nc, in_maps, CORE_IDS, **(_run_kwargs or {}))
    # core c returns tokens [128c, 128c+128) then [1024+128c, 1024+128c+128)
    out = np.empty((S, Dm), dtype=np.float32)
    for c in CORE_IDS:
        sh = res.results[c]["out"]
        out[128 * c:128 * c + 128] = sh[0:128]
        out[S // 2 + 128 * c:S // 2 + 128 * c + 128] = sh[128:256]
    out = out.reshape(B, S, Dm)
    if _run_kwargs:
        kernel.last_results = res
    return out
